# revision 1
# baseline (speedup 1.0000x reference)
"""MoE layer (B=4,S=2048,D=1024,F=2048,E=8,topK=2, softmax over token axis)
for 8 Trainium2 NeuronCores.

Strategy: expert parallelism with sparse token dispatch.
 - Host: gating matmul (jax-CPU for bit-exact selection), top-2, softmax over
   the token axis, per-expert token gather (+transpose to [D, C]).
 - Core e: dense FFN over only its ~2.1k routed tokens:
       hT = relu(W1[e].T-tiles @ xT + b1),  y = (hT.T @ W2[e]) * w_tok
   (two matmuls in f32r at full PE rate), output scaled by the per-token
   combine weight.
 - Host: scatter-add the 8 outputs back to [B,S,D].
"""
import os
import sys

for _p in ("/opt/trn_rl_repo", "/root/.axon_site/_ro/trn_rl_repo"):
    if os.path.isdir(_p) and _p not in sys.path:
        sys.path.append(_p)

import numpy as np
import concourse.bass as bass
import concourse.mybir as mybir
from concourse.tile import TileContext
from concourse.bass_utils import run_bass_kernel_spmd

B, S, D, F, E, K = 4, 2048, 1024, 2048, 8, 2
N = B * S
TB = 256            # token block
P = 128
DT = mybir.dt.float32r   # matmul operand dtype
NPDT = np.float32

_cache = {}


def _split_sync_waits(nc, max_waits=1):
    """The walrus build in this env rejects instructions carrying more than
    ~1 sync wait (Matmult S3_LW: 1; Drain: <3). Hoist extra waits onto
    same-engine NOPs placed immediately before the offending instruction —
    semantically identical (engine executes waits in order)."""
    ctr = 0
    for f in nc.m.functions:
        for blk in f.blocks:
            new_list = []
            changed = False
            for inst in blk.instructions:
                si = inst.sync_info
                ow = list(si.on_wait) if si and si.on_wait else []
                if len(ow) > max_waits:
                    extra, keep = ow[:-max_waits], ow[-max_waits:]
                    for i in range(0, len(extra), max_waits):
                        ctr += 1
                        nop = mybir.InstNoOp(
                            name=f"I-waitsplit-{ctr}",
                            engine=inst.engine,
                            sync_info=mybir.SyncInfo(
                                on_wait=list(extra[i:i + max_waits]), on_update=[]
                            ),
                        )
                        new_list.append(nop)
                    si.on_wait = keep
                    inst.sync_info = si
                    changed = True
                new_list.append(inst)
            if changed:
                blk.instructions = new_list


def _build(cpad):
    """Per-core FFN program over `cpad` routed tokens (zero-padded)."""
    nb = cpad // TB
    nc = bass.Bass("TRN2", target_bir_lowering=False, debug=False, num_devices=E)

    xT = nc.dram_tensor("xT", [D, cpad], DT, kind="ExternalInput")
    w1 = nc.dram_tensor("w1", [D, F], DT, kind="ExternalInput")
    w2 = nc.dram_tensor("w2", [F, D], DT, kind="ExternalInput")
    b1c = nc.dram_tensor("b1c", [P, F // P], mybir.dt.float32, kind="ExternalInput")
    wgtc = nc.dram_tensor("wgtc", [P, cpad // P], mybir.dt.float32, kind="ExternalInput")
    y = nc.dram_tensor("y", [cpad, D], mybir.dt.float32, kind="ExternalOutput")

    ND = D // P   # 8 d-tiles
    NF = F // P   # 16 f-tiles
    Relu = mybir.ActivationFunctionType.Relu
    Copy = mybir.ActivationFunctionType.Copy

    with TileContext(nc) as tc:
        with tc.tile_pool(name="wpool", bufs=1) as wpool, \
             tc.tile_pool(name="xpool", bufs=1) as xpool, \
             tc.tile_pool(name="hpool", bufs=1) as hpool, \
             tc.tile_pool(name="ypool", bufs=4) as ypool, \
             tc.tile_pool(name="ps1", bufs=4, space="PSUM") as ps1pool, \
             tc.tile_pool(name="ps2", bufs=4, space="PSUM") as ps2pool:

            # DMA issue order tuned so the PE starts ~2MB into the weight
            # stream instead of after 16MB: block-0 x first, then w1 in
            # quarter-F granularity (mm1's f-loop consumes fq=f//4 tiles in
            # order), then w2 (only needed once mm1 of block 0 finishes).
            # token blocks: 512-token super-blocks (mm1 rhs at N=512 issues
            # ~6% denser than N=256) + one 256 remainder if cpad % 512
            blocks = []
            off = 0
            while cpad - off >= 2 * TB:
                blocks.append((off, 2 * TB))
                off += 2 * TB
            if off < cpad:
                blocks.append((off, TB))

            # interleave block-0 x tiles with w1's first F-quarter so the
            # f=0 matmul chain's operands (xt0[d] + w1[d,fq0]) arrive first
            FQ = F // 4
            tb0 = blocks[0][1]
            w1_sb = {}
            xt0 = xpool.tile([P, ND * tb0], DT, tag="xt")
            for d in range(ND):
                nc.sync.dma_start(
                    out=xt0[:, d * tb0:(d + 1) * tb0],
                    in_=xT[d * P:(d + 1) * P, 0:tb0],
                )
                t = wpool.tile([P, FQ], DT, tag=f"w1_{d}_0")
                nc.sync.dma_start(out=t[:, :], in_=w1[d * P:(d + 1) * P, 0:FQ])
                w1_sb[(d, 0)] = t
            # warm-up: keep the PE busy during the initial weight DMA so the
            # HAM clock gate is at 8/8 (2.4GHz) when real matmuls start
            warm = wpool.tile([P, TB], DT, tag="warm")
            nc.gpsimd.memset(warm[:, :].bitcast(mybir.dt.float32), 0.0)
            ps_w = ps1pool.tile([P, TB], mybir.dt.float32, tag="ps1")
            for _ in range(24):
                nc.tensor.matmul(ps_w[:, :], lhsT=warm[:, :P], rhs=warm[:, :],
                                 start=True, stop=True)
            b1_sb = wpool.tile([P, F // P], mybir.dt.float32, tag="b1")
            nc.sync.dma_start(out=b1_sb[:, :], in_=b1c[:, :])
            wgt_sb = wpool.tile([P, cpad // P], mybir.dt.float32, tag="wgt")
            nc.sync.dma_start(out=wgt_sb[:, :], in_=wgtc[:, :])
            for fq in range(1, 4):
                for d in range(ND):
                    t = wpool.tile([P, FQ], DT, tag=f"w1_{d}_{fq}")
                    nc.sync.dma_start(
                        out=t[:, :], in_=w1[d * P:(d + 1) * P, fq * FQ:(fq + 1) * FQ]
                    )
                    w1_sb[(d, fq)] = t
            # w2 split by output-half (dh): mm2's (.,dh=0) chains only need
            # the first halves, so they unblock after 4MB instead of 8MB.
            w2_sb = {}
            for dh in range(2):
                for f in range(NF):
                    t = wpool.tile([P, D // 2], DT, tag=f"w2_{f}_{dh}")
                    nc.sync.dma_start(
                        out=t[:, :],
                        in_=w2[f * P:(f + 1) * P, dh * (D // 2):(dh + 1) * (D // 2)],
                    )
                    w2_sb[(f, dh)] = t

            for bi, (base, tb) in enumerate(blocks):
                if bi == 0:
                    xt = xt0
                else:
                    xt = xpool.tile([P, ND * tb], DT, tag="xt")
                    for d in range(ND):
                        nc.sync.dma_start(
                            out=xt[:, d * tb:(d + 1) * tb],
                            in_=xT[d * P:(d + 1) * P, base:base + tb],
                        )
                # mm1: hT[f*tb + t] = relu(sum_d w1_d[:,f].T @ xt_d + b1)
                hT = hpool.tile([P, NF * tb], DT, tag="hT")
                for f in range(NF):
                    ps = ps1pool.tile([P, tb], mybir.dt.float32, tag="ps1")
                    fq, fr = f // 4, f % 4
                    for d in range(ND):
                        nc.tensor.matmul(
                            ps[:, :],
                            lhsT=w1_sb[(d, fq)][:, fr * P:(fr + 1) * P],
                            rhs=xt[:, d * tb:(d + 1) * tb],
                            start=(d == 0),
                            stop=(d == ND - 1),
                        )
                    nc.scalar.activation(
                        hT[:, f * tb:(f + 1) * tb], ps[:, :], Relu,
                        bias=b1_sb[:, f:f + 1],
                    )
                # mm2: y[tok, :] = (hT.T @ w2) * wgt[tok]
                for dh in range(2):            # 512-wide halves of D (matches w2 arrival order)
                    for th in range(tb // P):  # 128-token subtiles of the block
                        ps2 = ps2pool.tile([P, D // 2], mybir.dt.float32, tag="ps2")
                        for f in range(NF):
                            nc.tensor.matmul(
                                ps2[:, :],
                                lhsT=hT[:, f * tb + th * P: f * tb + (th + 1) * P],
                                rhs=w2_sb[(f, dh)][:, :],
                                start=(f == 0),
                                stop=(f == NF - 1),
                            )
                        y_sb = ypool.tile([P, D // 2], mybir.dt.float32, tag="y")
                        nc.scalar.activation(
                            y_sb[:, :], ps2[:, :], Copy,
                            scale=wgt_sb[:, base // P + th: base // P + th + 1],
                        )
                        # store each quarter as soon as it is scaled so the
                        # final store doesn't serialize at the kernel tail
                        nc.sync.dma_start(
                            out=y[base + th * P: base + (th + 1) * P,
                                  dh * (D // 2):(dh + 1) * (D // 2)],
                            in_=y_sb[:, :],
                        )
    _split_sync_waits(nc)
    return nc


def _cpad(maxc):
    return max(TB, ((maxc + TB - 1) // TB) * TB)


def _routing(x_flat, gate_w):
    """Replicates: logits = x @ gate_w; top-2; softmax over token axis.
    Uses jax-CPU einsum when available so expert selection is bit-identical
    to the reference; falls back to float64 numpy."""
    try:
        import jax
        import jax.numpy as jnp
        cpu = jax.devices("cpu")[0]
        with jax.default_device(cpu):
            logits = np.asarray(
                jnp.einsum(
                    "bsd,de->bse",
                    jnp.asarray(x_flat.reshape(B, S, D)),
                    jnp.asarray(gate_w),
                )
            ).reshape(N, E)
    except Exception:
        logits = (x_flat.astype(np.float64) @ gate_w.astype(np.float64)).astype(
            np.float32
        )

    ar = np.arange(N)
    sel1 = logits.argmax(1)
    v1 = logits[ar, sel1]
    l2 = logits.copy()
    l2[ar, sel1] = -np.inf
    sel2 = l2.argmax(1)
    v2 = logits[ar, sel2]

    # softmax over the token axis per (batch, k) — matches jax.nn.softmax(axis=1)
    v = np.stack([v1, v2], 1).reshape(B, S, K)
    m = v.max(axis=1, keepdims=True)
    ev = np.exp(v - m)
    sm = (ev / ev.sum(axis=1, keepdims=True)).reshape(N, K).astype(np.float32)
    return sel1, sel2, sm[:, 0], sm[:, 1]


def kernel(x, gate_w, w1, b1, w2, b2):
    x = np.ascontiguousarray(np.asarray(x, dtype=np.float32))
    gate_w = np.ascontiguousarray(np.asarray(gate_w, dtype=np.float32))
    w1 = np.asarray(w1, dtype=np.float32)
    b1 = np.asarray(b1, dtype=np.float32)
    w2 = np.asarray(w2, dtype=np.float32)
    b2 = np.asarray(b2, dtype=np.float32)

    x_flat = x.reshape(N, D)
    sel1, sel2, sm1, sm2 = _routing(x_flat, gate_w)

    idx = []
    wgt = []
    for e in range(E):
        m1 = sel1 == e
        m2 = sel2 == e
        me = m1 | m2
        idx_e = np.nonzero(me)[0]
        wgt_e = np.where(m1[idx_e], sm1[idx_e], sm2[idx_e]).astype(np.float32)
        idx.append(idx_e)
        wgt.append(wgt_e)

    maxc = max(len(i) for i in idx)
    cpad = _cpad(maxc)

    if cpad not in _cache:
        _cache[cpad] = _build(cpad)
    nc = _cache[cpad]

    in_maps = []
    for e in range(E):
        c = len(idx[e])
        x_e = x_flat[idx[e]]                       # [c, D] contiguous row gather
        xT_e = np.zeros((D, cpad), dtype=NPDT)
        xT_e[:, :c] = x_e.T
        wgt_e = np.zeros(cpad, dtype=np.float32)
        wgt_e[:c] = wgt[e]
        in_maps.append({
            "xT": xT_e,
            "w1": np.ascontiguousarray(w1[e]),
            "w2": np.ascontiguousarray(w2[e]),
            "b1c": np.ascontiguousarray(b1[e].reshape(F // P, P).T),
            "wgtc": np.ascontiguousarray(wgt_e.reshape(cpad // P, P).T),
        })

    res = run_bass_kernel_spmd(nc, in_maps, list(range(E)))

    out = np.zeros((N, D), dtype=np.float32)
    for e in range(E):
        c = len(idx[e])
        out[idx[e]] += res.results[e]["y"][:c]
        if b2[e].any():
            out[idx[e]] += wgt[e][:, None] * b2[e][None, :]
    return out.reshape(B, S, D)


if __name__ == "__main__":
    rng = np.random.default_rng(0)
    inputs = {
        "x": rng.standard_normal((B, S, D)).astype(np.float32),
        "gate_w": (rng.standard_normal((D, E)) * 0.02).astype(np.float32),
        "w1": (rng.standard_normal((E, D, F)) * 0.02).astype(np.float32),
        "b1": np.zeros((E, F), np.float32),
        "w2": (rng.standard_normal((E, F, D)) * 0.02).astype(np.float32),
        "b2": np.zeros((E, D), np.float32),
    }
    out = kernel(**inputs)
    print("out", out.shape, out.dtype, np.abs(out).max())



# revision 3
# speedup vs baseline: 1.1482x; 1.1482x over previous
"""MoE layer (B=4,S=2048,D=1024,F=2048,E=8,topK=2, softmax over token axis)
for 8 Trainium2 NeuronCores.

Strategy: expert parallelism with sparse token dispatch, bf16 matmul operands.
 - Host: gating matmul (jax-CPU for bit-exact selection), top-2, softmax over
   the token axis, per-expert token gather (+transpose to [D, C], bf16).
 - Core e: dense FFN over only its ~2.2k routed tokens, exact token count
   (no 256-padding of compute):
       mm1: hT[f, tok] = relu(sum_d w1[d,f].T @ xT[d, tok] + b1[f])
       mm2: yT[dcol, tok] = sum_f w2[f, dcol].T @ hT[f, tok]
       scale: yT *= wgt[tok]  (DVE, token weight replicated on partitions)
   All matmuls bf16 operands / fp32 PSUM; moving dim = tokens, so both
   matmul phases cost exactly c rows per 128x128 output tile.
 - Host: transpose yT, scatter-add the 8 outputs back to [B,S,D], + b2.
"""
import os
import sys

for _p in ("/opt/trn_rl_repo", "/root/.axon_site/_ro/trn_rl_repo"):
    if os.path.isdir(_p) and _p not in sys.path:
        sys.path.append(_p)

import numpy as np
import ml_dtypes
import concourse.bass as bass
import concourse.mybir as mybir
from concourse.tile import TileContext
from concourse.bass_utils import run_bass_kernel_spmd

B, S, D, F, E, K = 4, 2048, 1024, 2048, 8, 2
N = B * S
P = 128
SB = 1024           # token superblock (2 PSUM chunks of 512)
DT = mybir.dt.bfloat16
NPDT = ml_dtypes.bfloat16
N_WARM = 10

_cache = {}


def _split_sync_waits(nc, max_waits=1):
    """The walrus build in this env rejects instructions carrying more than
    ~1 sync wait (Matmult S3_LW: 1; Drain: <3). Hoist extra waits onto
    same-engine NOPs placed immediately before the offending instruction —
    semantically identical (engine executes waits in order)."""
    ctr = 0
    for f in nc.m.functions:
        for blk in f.blocks:
            new_list = []
            changed = False
            for inst in blk.instructions:
                si = inst.sync_info
                ow = list(si.on_wait) if si and si.on_wait else []
                if len(ow) > max_waits:
                    extra, keep = ow[:-max_waits], ow[-max_waits:]
                    for i in range(0, len(extra), max_waits):
                        ctr += 1
                        nop = mybir.InstNoOp(
                            name=f"I-waitsplit-{ctr}",
                            engine=inst.engine,
                            sync_info=mybir.SyncInfo(
                                on_wait=list(extra[i:i + max_waits]), on_update=[]
                            ),
                        )
                        new_list.append(nop)
                    si.on_wait = keep
                    inst.sync_info = si
                    changed = True
                new_list.append(inst)
            if changed:
                blk.instructions = new_list


def _chunks(n):
    """Split n tokens into PSUM-bank chunks (<=512)."""
    out = []
    off = 0
    while n - off > 512:
        out.append((off, 512))
        off += 512
    out.append((off, n - off))
    return out


def _superblocks(c):
    out = []
    off = 0
    while c - off > SB:
        out.append((off, SB))
        off += SB
    out.append((off, c - off))
    return out


def _build(c):
    """Per-core FFN program over exactly `c` routed tokens (SPMD: all cores
    padded to the global max count)."""
    cpad = -(-c // P) * P
    nc = bass.Bass("TRN2", target_bir_lowering=False, debug=False, num_devices=E)

    xT = nc.dram_tensor("xT", [D, cpad], DT, kind="ExternalInput")
    w1 = nc.dram_tensor("w1", [D, F], DT, kind="ExternalInput")
    w2 = nc.dram_tensor("w2", [F, D], DT, kind="ExternalInput")
    b1c = nc.dram_tensor("b1c", [P, F // P], mybir.dt.float32, kind="ExternalInput")
    wgtb = nc.dram_tensor("wgtb", [P, cpad], mybir.dt.float32, kind="ExternalInput")
    yT = nc.dram_tensor("yT", [D, cpad], mybir.dt.float32, kind="ExternalOutput")

    ND = D // P    # 8 d-tiles (mm1 contraction / mm2 output tiles)
    NF = F // P    # 16 f-tiles
    FQ = F // 4    # w1 DMA quarter width (512)
    Relu = mybir.ActivationFunctionType.Relu
    sbs = _superblocks(c)

    with TileContext(nc) as tc:
        with tc.tile_pool(name="wpool", bufs=1) as wpool, \
             tc.tile_pool(name="xpool", bufs=1) as xpool, \
             tc.tile_pool(name="hpool", bufs=1) as hpool, \
             tc.tile_pool(name="ypool", bufs=4) as ypool, \
             tc.tile_pool(name="ps1", bufs=4, space="PSUM") as ps1pool, \
             tc.tile_pool(name="ps2", bufs=4, space="PSUM") as ps2pool:

            # ---- DMA issue order (single in-order HWDGE queue on sync) ----
            # sb0's x d-tiles interleaved with w1's first quarter so the
            # f=0 accumulation chain (x[d] + w1[d, f0]) can start ~1.5us in;
            # then the rest of w1 (f-major quarters), wgt/b1, w2, sb1+ x.
            sb0_off, sb0_len = sbs[0]
            x_sb = {}
            t = xpool.tile([P, ND * sb0_len], DT, tag="x_sb0")
            x_sb[0] = t
            w1_sb = {}
            for d in range(ND):
                nc.sync.dma_start(
                    out=t[:, d * sb0_len:(d + 1) * sb0_len],
                    in_=xT[d * P:(d + 1) * P, sb0_off:sb0_off + sb0_len],
                )
                w = wpool.tile([P, FQ], DT, tag=f"w1_{d}_0")
                nc.sync.dma_start(out=w[:, :], in_=w1[d * P:(d + 1) * P, 0:FQ])
                w1_sb[(d, 0)] = w

            # warm-up: keep the PE busy during the initial DMA so the HAM
            # clock gate is released by the time real matmuls start
            warm = wpool.tile([P, 256], DT, tag="warm")
            nc.vector.memzero(warm[:, :].bitcast(mybir.dt.float32))
            ps_w = ps1pool.tile([P, 512], mybir.dt.float32, tag="ps1")
            for _ in range(N_WARM):
                nc.tensor.matmul(ps_w[:, :256], lhsT=warm[:, :P], rhs=warm[:, :],
                                 start=True, stop=True)

            b1_sb = wpool.tile([P, F // P], mybir.dt.float32, tag="b1")
            nc.sync.dma_start(out=b1_sb[:, :], in_=b1c[:, :])
            wgt_sb = wpool.tile([P, cpad], mybir.dt.float32, tag="wgt")
            nc.sync.dma_start(out=wgt_sb[:, :], in_=wgtb[:, :])
            for fq in range(1, 4):
                for d in range(ND):
                    w = wpool.tile([P, FQ], DT, tag=f"w1_{d}_{fq}")
                    nc.sync.dma_start(
                        out=w[:, :], in_=w1[d * P:(d + 1) * P, fq * FQ:(fq + 1) * FQ]
                    )
                    w1_sb[(d, fq)] = w
            w2_sb = {}
            for f in range(NF):
                w = wpool.tile([P, D], DT, tag=f"w2_{f}")
                nc.sync.dma_start(out=w[:, :], in_=w2[f * P:(f + 1) * P, :])
                w2_sb[f] = w
            for si, (off, ln) in enumerate(sbs):
                if si == 0:
                    continue
                t = xpool.tile([P, ND * ln], DT, tag=f"x_sb{si}")
                x_sb[si] = t
                for d in range(ND):
                    nc.sync.dma_start(
                        out=t[:, d * ln:(d + 1) * ln],
                        in_=xT[d * P:(d + 1) * P, off:off + ln],
                    )

            # ---- compute: per superblock, mm1 then mm2 ----
            for si, (off, ln) in enumerate(sbs):
                chs = _chunks(ln)
                hT = hpool.tile([P, NF * ln], DT, tag=f"hT{si}")
                # mm1: hT[f, tok] = relu(sum_d w1[d,f].T @ x[d, tok] + b1[f])
                for f in range(NF):
                    fq, fr = f // 4, f % 4
                    for (co, cl) in chs:
                        ps = ps1pool.tile([P, 512], mybir.dt.float32, tag="ps1")
                        for d in range(ND):
                            nc.tensor.matmul(
                                ps[:, :cl],
                                lhsT=w1_sb[(d, fq)][:, fr * P:(fr + 1) * P],
                                rhs=x_sb[si][:, d * ln + co: d * ln + co + cl],
                                start=(d == 0),
                                stop=(d == ND - 1),
                            )
                        nc.scalar.activation(
                            hT[:, f * ln + co: f * ln + co + cl],
                            ps[:, :cl], Relu,
                            bias=b1_sb[:, f:f + 1],
                        )
                # mm2: yT[dcol, tok] = (sum_f w2[f, dcol].T @ hT[f, tok]) * wgt
                for dt in range(ND):
                    for (co, cl) in chs:
                        ps = ps2pool.tile([P, 512], mybir.dt.float32, tag="ps2")
                        for f in range(NF):
                            nc.tensor.matmul(
                                ps[:, :cl],
                                lhsT=w2_sb[f][:, dt * P:(dt + 1) * P],
                                rhs=hT[:, f * ln + co: f * ln + co + cl],
                                start=(f == 0),
                                stop=(f == NF - 1),
                            )
                        y_sb = ypool.tile([P, 512], mybir.dt.float32, tag="y")
                        nc.vector.tensor_mul(
                            y_sb[:, :cl], ps[:, :cl],
                            wgt_sb[:, off + co: off + co + cl],
                        )
                        nc.scalar.dma_start(
                            out=yT[dt * P:(dt + 1) * P, off + co: off + co + cl],
                            in_=y_sb[:, :cl],
                        )
    _split_sync_waits(nc)
    return nc


def _routing(x_flat, gate_w):
    """Replicates: logits = x @ gate_w; top-2; softmax over token axis.
    Uses jax-CPU einsum when available so expert selection is bit-identical
    to the reference; falls back to float64 numpy."""
    try:
        import jax
        import jax.numpy as jnp
        cpu = jax.devices("cpu")[0]
        with jax.default_device(cpu):
            logits = np.asarray(
                jnp.einsum(
                    "bsd,de->bse",
                    jnp.asarray(x_flat.reshape(B, S, D)),
                    jnp.asarray(gate_w),
                )
            ).reshape(N, E)
    except Exception:
        logits = (x_flat.astype(np.float64) @ gate_w.astype(np.float64)).astype(
            np.float32
        )

    ar = np.arange(N)
    sel1 = logits.argmax(1)
    v1 = logits[ar, sel1]
    l2 = logits.copy()
    l2[ar, sel1] = -np.inf
    sel2 = l2.argmax(1)
    v2 = logits[ar, sel2]

    # softmax over the token axis per (batch, k) — matches jax.nn.softmax(axis=1)
    v = np.stack([v1, v2], 1).reshape(B, S, K)
    m = v.max(axis=1, keepdims=True)
    ev = np.exp(v - m)
    sm = (ev / ev.sum(axis=1, keepdims=True)).reshape(N, K).astype(np.float32)
    return sel1, sel2, sm[:, 0], sm[:, 1]


def _dispatch(inputs):
    """Host routing + per-core input prep. Returns (nc, in_maps, idx, wgt, c)."""
    x = np.ascontiguousarray(np.asarray(inputs["x"], dtype=np.float32))
    gate_w = np.ascontiguousarray(np.asarray(inputs["gate_w"], dtype=np.float32))
    w1 = np.asarray(inputs["w1"], dtype=np.float32)
    b1 = np.asarray(inputs["b1"], dtype=np.float32)
    w2 = np.asarray(inputs["w2"], dtype=np.float32)

    x_flat = x.reshape(N, D)
    sel1, sel2, sm1, sm2 = _routing(x_flat, gate_w)

    idx = []
    wgt = []
    for e in range(E):
        m1 = sel1 == e
        m2 = sel2 == e
        idx_e = np.nonzero(m1 | m2)[0]
        wgt_e = np.where(m1[idx_e], sm1[idx_e], sm2[idx_e]).astype(np.float32)
        idx.append(idx_e)
        wgt.append(wgt_e)

    c = max(len(i) for i in idx)
    cpad = -(-c // P) * P

    if c not in _cache:
        _cache[c] = _build(c)
    nc = _cache[c]

    in_maps = []
    for e in range(E):
        ce = len(idx[e])
        xT_e = np.zeros((D, cpad), dtype=NPDT)
        xT_e[:, :ce] = x_flat[idx[e]].T.astype(NPDT)
        wgt_e = np.zeros(cpad, dtype=np.float32)
        wgt_e[:ce] = wgt[e]
        in_maps.append({
            "xT": xT_e,
            "w1": np.ascontiguousarray(w1[e].astype(NPDT)),
            "w2": np.ascontiguousarray(w2[e].astype(NPDT)),
            "b1c": np.ascontiguousarray(b1[e].reshape(F // P, P).T),
            "wgtb": np.ascontiguousarray(
                np.broadcast_to(wgt_e[None, :], (P, cpad))),
        })
    return nc, in_maps, idx, wgt, c


def kernel(x, gate_w, w1, b1, w2, b2):
    inputs = {"x": x, "gate_w": gate_w, "w1": w1, "b1": b1, "w2": w2}
    nc, in_maps, idx, wgt, c = _dispatch(inputs)
    b2 = np.asarray(b2, dtype=np.float32)

    res = run_bass_kernel_spmd(nc, in_maps, list(range(E)))

    out = np.zeros((N, D), dtype=np.float32)
    for e in range(E):
        ce = len(idx[e])
        out[idx[e]] += res.results[e]["yT"][:, :ce].T
        if b2[e].any():
            out[idx[e]] += wgt[e][:, None] * b2[e][None, :]
    return out.reshape(B, S, D)


if __name__ == "__main__":
    rng = np.random.default_rng(0)
    inputs = {
        "x": rng.standard_normal((B, S, D)).astype(np.float32),
        "gate_w": (rng.standard_normal((D, E)) * 0.02).astype(np.float32),
        "w1": (rng.standard_normal((E, D, F)) * 0.02).astype(np.float32),
        "b1": np.zeros((E, F), np.float32),
        "w2": (rng.standard_normal((E, F, D)) * 0.02).astype(np.float32),
        "b2": np.zeros((E, D), np.float32),
    }
    out = kernel(**inputs)
    print("out", out.shape, out.dtype, np.abs(out).max())
